# revision 1
# baseline (speedup 1.0000x reference)
"""BEiT attention block (dense_transformer) as a Trainium2 Bass/Tile kernel.

Sharding: head-parallel across 8 NeuronCores. Core c owns heads {2c, 2c+1}
(= qkv channels c*128 .. c*128+127). Each core computes its heads' QKV,
attention with relative-position bias, and a partial projection
out_partial = O_heads @ proj_weight[:, c*128:(c+1)*128].T, returned
transposed as [1024, 4100] bf16. Host sums the 8 partials + proj bias.

Device-side design notes:
  - QT/KT/VT computed in [channel, seq] layout (weights stationary, xT moving)
  - attention scores computed transposed: S[k, q] = K @ Q^T per (batch, head)
  - softmax without max subtraction (logits bounded ~±4); rel-pos bias applied
    multiplicatively: P = exp(S) * expB, with expB = exp(bias) precomputed
    on host (exp(s+b) = exp(s)exp(b)); padded keys have expB = 0.
  - softmax denominators ride as a ones-column in the V_ext stationary operand
    of the PV matmul (row 64 of O^T accumulates sum_k P[k, q]).
  - normalization deferred: 1/sums broadcast across partitions via a DRAM
    round trip, applied when moving O^T into the projection input buffer.
  - q dim per batch is 1025 = 2*512 + 1: the 1025th query column is handled
    by a separate "tail" path batched in single psum banks (S_C / OT_C).
"""

import os
import sys
import numpy as np

for _p in ("/opt/trn_rl_repo", "/root/.axon_site/_ro/trn_rl_repo"):
    if os.path.isdir(_p) and _p not in sys.path:
        sys.path.insert(0, _p)

import ml_dtypes
from contextlib import ExitStack

import concourse.bass as bass
import concourse.mybir as mybir
import concourse.tile as tile
from concourse import bacc
from concourse.bass_utils import run_bass_kernel_spmd

BF16NP = ml_dtypes.bfloat16
F32 = mybir.dt.float32
BF = mybir.dt.bfloat16

# Problem constants (hardcoded per spec)
B, N, C = 4, 1025, 1024
NH, HD = 16, 64
NCORES = 8
HPC = 2                      # heads per core
BN = B * N                   # 4100
SEQP = 1152                  # per-batch padded seq length (9*128)
KT = 9                       # key tiles (of 128) per batch
NQM = 1024                   # "main" query columns; col 1024 is the tail
PATCH = 16
OLD_WS = (24, 24)
NEW_WS = (32, 32)
VBLK = 80                    # V_ext block stride (64 V cols + 1 ones + pad)

_CACHE = {}


# ----------------------------------------------------------------------------
# host-side: relative position bias (matches reference bit-for-bit-ish)
# ----------------------------------------------------------------------------

def _gen_relative_position_index(window_size):
    wh, ww = window_size
    num_rel = (2 * wh - 1) * (2 * ww - 1) + 3
    coords = np.stack(np.meshgrid(np.arange(wh), np.arange(ww), indexing='ij'))
    cf = coords.reshape(2, -1)
    rel = cf[:, :, None] - cf[:, None, :]
    rel = rel.transpose(1, 2, 0).astype(np.int64)
    rel[:, :, 0] += wh - 1
    rel[:, :, 1] += ww - 1
    rel[:, :, 0] *= 2 * ww - 1
    n = wh * ww + 1
    rpi = np.zeros((n, n), dtype=np.int64)
    rpi[1:, 1:] = rel.sum(-1)
    rpi[0, 0:] = num_rel - 3
    rpi[0:, 0] = num_rel - 2
    rpi[0, 0] = num_rel - 1
    return rpi


def _rel_pos_bias(table):
    """table [2212, 16] fp32 -> bias [nH, N, N] fp32 (same math as reference)."""
    import jax
    import jax.numpy as jnp

    oh, ow = 2 * OLD_WS[0] - 1, 2 * OLD_WS[1] - 1
    nh_, nw = 2 * NEW_WS[0] - 1, 2 * NEW_WS[1] - 1
    old_num = oh * ow + 3
    new_num = nh_ * nw + 3
    with jax.default_device(jax.devices("cpu")[0]):
        t = jnp.asarray(table)
        sub = t[: old_num - 3].reshape(ow, oh, NH).transpose(2, 0, 1)
        sub = jax.image.resize(sub, (NH, nh_, nw), method='bilinear')
        sub = sub.transpose(1, 2, 0).reshape(new_num - 3, NH)
        new_table = np.asarray(jnp.concatenate([sub, t[old_num - 3:]], axis=0))
    idx = _gen_relative_position_index(NEW_WS)
    bias = new_table[idx.reshape(-1)].reshape(N, N, NH)  # [q, k, h]
    return bias.transpose(2, 0, 1)  # [h, q, k]


# ----------------------------------------------------------------------------
# device kernel
# ----------------------------------------------------------------------------

def _bn_chunks(width=512, total=BN):
    out = []
    o = 0
    while o < total:
        out.append((o, min(width, total - o)))
        o += width
    return out


def _pad_pieces(c0, cw):
    """Map a BN column range [c0, c0+cw) to (padded_dst, src_off, width) pieces
    split at batch boundaries. Padded layout has per-batch stride SEQP."""
    pieces = []
    o = c0
    while o < c0 + cw:
        b = o // N
        loc = o - b * N
        w = min(c0 + cw - o, N - loc)
        pieces.append((b * SEQP + loc, o - c0, w))
        o += w
    return pieces


def build_nc(repeat=1):
    nc = bacc.Bacc("TRN2", target_bir_lowering=False, debug=False)

    xT = nc.dram_tensor("xT", [C, BN], BF, kind="ExternalInput").ap()
    wqT = nc.dram_tensor("wqT", [C, 128], BF, kind="ExternalInput").ap()
    wkT = nc.dram_tensor("wkT", [C, 128], BF, kind="ExternalInput").ap()
    wvT = nc.dram_tensor("wvT", [C, 128], BF, kind="ExternalInput").ap()
    qb = nc.dram_tensor("qb", [128, 1], F32, kind="ExternalInput").ap()
    kb = nc.dram_tensor("kb", [128, 1], F32, kind="ExternalInput").ap()
    vb = nc.dram_tensor("vb", [128, 1], F32, kind="ExternalInput").ap()
    pwT = nc.dram_tensor("pwT", [128, C], BF, kind="ExternalInput").ap()
    expb = nc.dram_tensor("expb", [128, HPC * KT * N], BF, kind="ExternalInput").ap()
    outt = nc.dram_tensor("out_t", [C, BN], BF, kind="ExternalOutput").ap()
    recs = nc.dram_tensor("recip_scratch", [B * HPC, N], F32).ap()

    EXP = mybir.ActivationFunctionType.Exp

    with TileCtx(nc) as (tc, ctx):
        singles = ctx.enter_context(tc.tile_pool(name="singles", bufs=1))

        # persistent SBUF state (optionally per-batch tiles so attention(b)
        # only depends on QKV(b), and proj(b) only on attention(b))
        if os.environ.get("KV_SPLIT", "batch") == "batch":
            qt_sb = [singles.tile([128, SEQP], BF, name=f"qt_sb{b}") for b in range(B)]
            kt_sb = [singles.tile([128, SEQP], BF, name=f"kt_sb{b}") for b in range(B)]
            ve_sb = [singles.tile([128, KT * HPC * VBLK], BF, name=f"ve_sb{b}") for b in range(B)]
            otall_sb = [singles.tile([128, N], BF, name=f"otall_sb{b}") for b in range(B)]
        else:
            _qt = singles.tile([128, B * SEQP], BF, name="qt_sb")
            _kt = singles.tile([128, B * SEQP], BF, name="kt_sb")
            _ve = singles.tile([128, B * KT * HPC * VBLK], BF, name="ve_sb")
            _ot = singles.tile([128, BN], BF, name="otall_sb")
            qt_sb = [_qt[:, b * SEQP:(b + 1) * SEQP] for b in range(B)]
            kt_sb = [_kt[:, b * SEQP:(b + 1) * SEQP] for b in range(B)]
            ve_sb = [_ve[:, b * KT * HPC * VBLK:(b + 1) * KT * HPC * VBLK] for b in range(B)]
            otall_sb = [_ot[:, b * N:(b + 1) * N] for b in range(B)]
        expb_sb = singles.tile([128, HPC * KT * N], BF, name="expb_sb")
        pw_sb = singles.tile([128, C], BF, name="pw_sb")
        qb_sb = singles.tile([128, 1], F32, name="qb_sb")
        kb_sb = singles.tile([128, 1], F32, name="kb_sb")
        vb_sb = singles.tile([128, 1], F32, name="vb_sb")
        ident_sb = singles.tile([128, 128], BF, name="ident_sb")

        _ms = nc.gpsimd.memset if os.environ.get("KV_MEMSET", "gpsimd") == "gpsimd" else nc.vector.memset
        for b in range(B):
            _ms(qt_sb[b], 0.0)
            _ms(kt_sb[b], 0.0)
            _ms(ve_sb[b], 1.0)
        from concourse.masks import make_identity
        make_identity(nc, ident_sb)

        nc.sync.dma_start(out=expb_sb, in_=expb)
        nc.sync.dma_start(out=pw_sb, in_=pwT)
        nc.sync.dma_start(out=qb_sb, in_=qb)
        nc.sync.dma_start(out=kb_sb, in_=kb)
        nc.sync.dma_start(out=vb_sb, in_=vb)

        # weight tiles [128, 128] per C-chunk
        wpool = ctx.enter_context(tc.tile_pool(name="weights", bufs=1))
        wq_t, wk_t, wv_t = [], [], []
        for kc in range(8):
            for lst, src, nm in ((wq_t, wqT, "wq"), (wk_t, wkT, "wk"), (wv_t, wvT, "wv")):
                t = wpool.tile([128, 128], BF, name=f"{nm}{kc}")
                nc.sync.dma_start(out=t, in_=src[kc * 128:(kc + 1) * 128, :])
                lst.append(t)

        for _rep in range(repeat):
            _emit_phases(nc, tc, qt_sb, kt_sb, ve_sb, expb_sb, otall_sb,
                         pw_sb, qb_sb, kb_sb, vb_sb, ident_sb, wq_t, wk_t, wv_t,
                         xT, recs, outt)

    nc.compile()
    return nc


def _emit_phases(nc, tc, qt_sb, kt_sb, ve_sb, expb_sb, otall_sb,
                 pw_sb, qb_sb, kb_sb, vb_sb, ident_sb, wq_t, wk_t, wv_t,
                 xT, recs, outt):
    EXP = mybir.ActivationFunctionType.Exp
    if True:
        # ------------------------- QKV phase (batch-local) -------------------
        # Q/K/V all in [chan, seq] via W stationary (big streams, few weight
        # loads); V_ext blocks produced by PE transposes of 128-col blocks.
        with tc.tile_pool(name="xin", bufs=3) as xpool, \
             tc.tile_pool(name="vtmp", bufs=2) as vtpool, \
             tc.tile_pool(name="qkv_ps", bufs=2, space="PSUM") as qkps, \
             tc.tile_pool(name="tp_ps", bufs=2, space="PSUM") as tppool:
            for b in range(B):
                xb = []
                for kc in range(8):
                    xt = xpool.tile([128, N], BF, tag=f"xt{kc}")
                    nc.sync.dma_start(out=xt, in_=xT[kc * 128:(kc + 1) * 128, b * N:(b + 1) * N])
                    xb.append(xt)
                vt_b = vtpool.tile([128, N], BF, tag="vt")
                for (w_t, dst, bias_col, tg) in ((wq_t, qt_sb[b], qb_sb, "q"),
                                                 (wk_t, kt_sb[b], kb_sb, "k"),
                                                 (wv_t, vt_b, vb_sb, "v")):
                    for (c0, cw) in ((0, 512), (512, 512), (1024, 1)):
                        ps = qkps.tile([128, 512], F32, tag=tg)
                        for kc in range(8):
                            nc.tensor.matmul(ps[:, :cw], w_t[kc], xb[kc][:, c0:c0 + cw],
                                             start=(kc == 0), stop=(kc == 7))
                        nc.vector.tensor_scalar_add(dst[:, c0: c0 + cw],
                                                    ps[:, :cw], bias_col)
                for kt in range(KT):
                    stw = 128 if kt < 8 else 1
                    vp = tppool.tile([128, 128], BF, tag="tp")
                    nc.tensor.transpose(vp[:stw, :], vt_b[:, kt * 128: kt * 128 + stw],
                                        ident_sb)
                    # both heads' V in one copy: dst cols {0..63} u {VBLK..VBLK+63}
                    vdst = ve_sb[b][:stw, kt * HPC * VBLK: kt * HPC * VBLK + VBLK + 64]
                    vdst = bass.AP(tensor=vdst.tensor, offset=vdst.offset,
                                   ap=list(vdst.ap[:-1]) + [[VBLK, 2], [1, 64]])
                    vsrc = vp[:stw, :]
                    vsrc = bass.AP(tensor=vsrc.tensor, offset=vsrc.offset,
                                   ap=list(vsrc.ap[:-1]) + [[64, 2], [1, 64]])
                    nc.vector.tensor_copy(vdst, vsrc)

        # ------------------------- attention phase -------------------------
        with tc.tile_pool(name="s_ps", bufs=2, space="PSUM") as sps, \
             tc.tile_pool(name="ot_ps", bufs=1, space="PSUM") as otps, \
             tc.tile_pool(name="tail_ps", bufs=2, space="PSUM") as tailps, \
             tc.tile_pool(name="praw", bufs=3) as prawpool, \
             tc.tile_pool(name="pmul", bufs=4) as ppool, \
             tc.tile_pool(name="ptail", bufs=2) as ptpool, \
             tc.tile_pool(name="otsb", bufs=3) as otsbpool, \
             tc.tile_pool(name="rr", bufs=3) as rrpool, \
             tc.tile_pool(name="rbc", bufs=3) as rbcpool:


            for u in range(B * HPC):
                b, h = u // HPC, u % HPC
                hp = h * 64

                def kt_lhs(kt):
                    return kt_sb[b][hp:hp + 64, kt * 128:(kt + 1) * 128]

                def ve_lhs(kt):
                    blk = (kt * HPC + h) * VBLK
                    return ve_sb[b][:, blk: blk + 65]

                # tail query column (q = 1024): rotating S-pool slot, released
                # by exp_tail before the main kt loop needs both S slots
                ot_c = tailps.tile([65, 1], F32, tag="otc")
                s_tail = sps.tile([128, KT], F32, tag="s")
                for kt in range(KT):
                    nc.tensor.matmul(s_tail[:, kt:kt + 1], kt_lhs(kt),
                                     qt_sb[b][hp:hp + 64, NQM: NQM + 1],
                                     start=True, stop=True)
                ptraw = ptpool.tile([128, KT], BF, tag="ptraw")
                nc.scalar.activation(ptraw, s_tail, EXP)
                ptm = ptpool.tile([128, KT], BF, tag="ptm")
                eb_tail = bass.AP(tensor=expb_sb.tensor,
                                  offset=expb_sb.offset + h * KT * N + NQM,
                                  ap=[expb_sb.ap[0], [N, KT]])
                nc.vector.tensor_mul(ptm, ptraw, eb_tail)

                ot = otps.tile([65, NQM], F32, tag="ot")
                for kt in range(KT):
                    s = sps.tile([128, NQM], F32, tag="s")
                    nc.tensor.matmul(s[:, 0:512], kt_lhs(kt),
                                     qt_sb[b][hp:hp + 64, 0:512], start=True, stop=True)
                    nc.tensor.matmul(s[:, 512:1024], kt_lhs(kt),
                                     qt_sb[b][hp:hp + 64, 512:1024], start=True, stop=True)
                    praw = prawpool.tile([128, NQM], BF, tag="praw")
                    nc.scalar.activation(praw, s, EXP)
                    p = ppool.tile([128, NQM], BF, tag="p")
                    mul_eng = nc.gpsimd if kt % 3 == 2 else nc.vector
                    mul_eng.tensor_mul(p, praw,
                                       expb_sb[:, (h * KT + kt) * N:(h * KT + kt) * N + NQM])
                    nc.tensor.matmul(ot[:, 0:512], ve_lhs(kt), p[:, 0:512],
                                     start=(kt == 0), stop=(kt == KT - 1))
                    nc.tensor.matmul(ot[:, 512:1024], ve_lhs(kt), p[:, 512:1024],
                                     start=(kt == 0), stop=(kt == KT - 1))
                    nc.tensor.matmul(ot_c, ve_lhs(kt), ptm[:, kt:kt + 1],
                                     start=(kt == 0), stop=(kt == KT - 1))

                ot_sb = otsbpool.tile([65, N], F32, tag="otsb")
                nc.vector.tensor_copy(ot_sb[:, 0:NQM], ot)
                nc.vector.tensor_copy(ot_sb[:, NQM:N], ot_c)
                rr = rrpool.tile([1, N], F32, tag="rr")
                nc.vector.reciprocal(rr, ot_sb[64:65, :])
                nc.sync.dma_start(out=recs[u:u + 1, :], in_=rr)
                rbc = rbcpool.tile([64, N], F32, tag="rbc")
                nc.sync.dma_start(out=rbc, in_=bass.AP(tensor=recs.tensor,
                                                       offset=recs.offset + u * N,
                                                       ap=[[0, 64], [1, N]]))
                nc.gpsimd.tensor_mul(otall_sb[b][hp:hp + 64, :],
                                     ot_sb[0:64, :], rbc)

        # ------------------------- projection phase -------------------------
        with tc.tile_pool(name="pj_ps", bufs=3, space="PSUM") as pjps, \
             tc.tile_pool(name="osb", bufs=2) as opool:
            for ct in range(8):
                osb = opool.tile([128, BN], BF, tag="osb")
                ci = 0
                for b in range(B):
                    for (q0, qw) in ((0, 512), (512, 512), (1024, 1)):
                        pj = pjps.tile([128, 512], F32, tag="pj")
                        nc.tensor.matmul(pj[:, :qw], pw_sb[:, ct * 128:(ct + 1) * 128],
                                         otall_sb[b][:, q0:q0 + qw], start=True, stop=True)
                        if ci % 2 == 0:
                            nc.scalar.copy(osb[:, b * N + q0: b * N + q0 + qw], pj[:, :qw])
                        else:
                            nc.vector.tensor_copy(osb[:, b * N + q0: b * N + q0 + qw], pj[:, :qw])
                        ci += 1
                nc.sync.dma_start(out=outt[ct * 128:(ct + 1) * 128, :], in_=osb)


class TileCtx:
    """with TileCtx(nc) as (tc, ctx): ... (TileContext + ExitStack combined)."""

    def __init__(self, nc):
        self.nc = nc

    def __enter__(self):
        self._tc = tile.TileContext(self.nc)
        self._ctx = ExitStack()
        tc = self._tc.__enter__()
        ctx = self._ctx.__enter__()
        return tc, ctx

    def __exit__(self, *exc):
        self._ctx.__exit__(*exc)
        return self._tc.__exit__(*exc)


# ----------------------------------------------------------------------------
# host-side input prep / output gather
# ----------------------------------------------------------------------------

def _prep_inputs(x, qkv_weight, q_bias, k_bias, v_bias, proj_weight, rel_pos_table):
    """Returns in_maps (list of 8 dicts)."""
    scale = (C // NH) ** -0.5  # 0.125

    xT = np.ascontiguousarray(
        np.asarray(x, dtype=np.float32).reshape(BN, C).T).astype(BF16NP)

    bias = _rel_pos_bias(np.asarray(rel_pos_table, dtype=np.float32))  # [h, q, k]
    ebT = np.exp(bias.transpose(0, 2, 1).astype(np.float32))           # [h, k, q]
    full = np.zeros((NH, KT * 128, N), dtype=np.float32)
    full[:, :N, :] = ebT
    # [h, kt, p, q] -> per head [p, kt*q]
    full = full.reshape(NH, KT, 128, N)

    qkv_w = np.asarray(qkv_weight, dtype=np.float32)
    qb_full = np.asarray(q_bias, dtype=np.float32)
    kb_full = np.asarray(k_bias, dtype=np.float32)
    vb_full = np.asarray(v_bias, dtype=np.float32)
    pw = np.asarray(proj_weight, dtype=np.float32)

    in_maps = []
    for c in range(NCORES):
        sl = slice(c * 128, (c + 1) * 128)
        wq = (qkv_w[0 * C:1 * C][sl] * scale).T
        wk = qkv_w[1 * C:2 * C][sl].T
        wv = qkv_w[2 * C:3 * C][sl].T
        eb_core = full[2 * c:2 * c + 2]                    # [2, kt, p, q]
        eb_dev = np.ascontiguousarray(
            eb_core.transpose(2, 0, 1, 3).reshape(128, HPC * KT * N)).astype(BF16NP)
        in_maps.append({
            "xT": xT,
            "wqT": np.ascontiguousarray(wq).astype(BF16NP),
            "wkT": np.ascontiguousarray(wk).astype(BF16NP),
            "wvT": np.ascontiguousarray(wv).astype(BF16NP),
            "qb": np.ascontiguousarray((qb_full[sl] * scale).reshape(128, 1)),
            "kb": np.ascontiguousarray(kb_full[sl].reshape(128, 1)),
            "vb": np.ascontiguousarray(vb_full[sl].reshape(128, 1)),
            "pwT": np.ascontiguousarray(pw[:, sl].T).astype(BF16NP),
            "expb": eb_dev,
        })
    return in_maps


LAST_RESULTS = None


def kernel(x, qkv_weight, q_bias, k_bias, v_bias, proj_weight, proj_bias,
           rel_pos_table, res_h=512, res_w=512):
    global LAST_RESULTS
    if "nc" not in _CACHE:
        _CACHE["nc"] = build_nc()
    nc = _CACHE["nc"]

    in_maps = _prep_inputs(x, qkv_weight, q_bias, k_bias, v_bias, proj_weight,
                           rel_pos_table)
    trace = os.environ.get("KERNEL_TRACE", "0") == "1"
    res = run_bass_kernel_spmd(nc, in_maps, core_ids=list(range(NCORES)),
                               trace=trace)
    LAST_RESULTS = res

    total = np.zeros((C, BN), dtype=np.float32)
    for r in res.results:
        total += np.asarray(r["out_t"], dtype=np.float32)
    out = total.T + np.asarray(proj_bias, dtype=np.float32)
    return np.ascontiguousarray(out.reshape(B, N, C), dtype=np.float32)

